# revision 8
# baseline (speedup 1.0000x reference)
"""Fused LN + multi-head attention block for Trainium2, data-parallel over 8 NeuronCores.

Problem (hardcoded): B=16, N=1024, EMB=128, H=8, INNER=1024, fp32 I/O.
Each core handles 2 batches; no cross-core communication is needed.

Per-core pipeline (all matmuls in bf16, accumulation fp32 in PSUM):
  1. LayerNorm in token-major tiles, TensorE-transpose -> xT[emb, 2048] bf16
     (gamma/beta folded into the transpose-PSUM evacuation).
  2. qT/kT[d, t] per head via w-as-lhsT matmuls; v token-major [t, (h, d+1)]
     with a constant ones column appended per head.
  3. Scores transposed: ST[j, i] = sum_d k[j,d] q[i,d]; exp on ScalarE with
     the 1/sqrt(INNER) scale folded into the activation scale. No
     max-subtraction: scores are ~N(0, 0.35) by construction, exp is safe.
  4. PV with exp(ST) tiles as weights: out[i, (d, Z)] = E^T @ [v | 1]; the
     ones column yields the softmax denominator Z_i in column 128 for free.
     Normalization = per-partition tensor_scalar multiply by 1/Z during the
     PSUM evacuation.
  5. TensorE-transpose attn -> [d, i], project with w_proj chunks as lhsT
     accumulating heads in PSUM, add bias, transpose back to token-major,
     DMA out.
"""

import sys

for _p in ("/opt/trn_rl_repo",):
    if _p not in sys.path:
        sys.path.insert(0, _p)

import numpy as np

import concourse.bass as bass
import concourse.mybir as mybir
import concourse.tile as tile
from concourse.masks import make_identity
from concourse.bass_utils import run_bass_kernel_spmd

F32 = mybir.dt.float32
BF16 = mybir.dt.bfloat16
ALU = mybir.AluOpType
AFT = mybir.ActivationFunctionType

N_CORES = 8
B = 16
N = 1024
EMB = 128
H = 8
D = 128
INNER = EMB * H
B_LOC = B // N_CORES          # 2 batches per core
T = B_LOC * N                 # 2048 tokens per core
NT = T // 128                 # 16 token tiles per core
NT_B = N // 128               # 8 token tiles per batch
SCALE = float(INNER) ** -0.5  # 1/32, folded into exp()
EPS = 1e-5


# ---------------------------------------------------------------------------
# Workaround: this walrus build rejects instructions carrying more than a
# couple of embedded semaphore waits ("Too many sync wait commands"). After
# Tile scheduling, split excess waits onto standalone same-engine NoOps
# placed immediately before the instruction (engine program order preserves
# the blocking semantics).
def split_sync_waits(nc, max_waits=1):
    n_split = 0
    for f in nc.m.functions:
        for bb in f.blocks:
            new_insts = []
            for inst in bb.instructions:
                si = getattr(inst, "sync_info", None)
                waits = list(si.on_wait) if (si is not None and si.on_wait) else []
                if len(waits) > max_waits:
                    keep = waits[:max_waits]
                    extra = waits[max_waits:]
                    for k, w in enumerate(extra):
                        nop = mybir.InstNoOp(
                            name=f"{inst.name}-wsplit{k}",
                            sync_info=mybir.SyncInfo(on_wait=[w], on_update=[]),
                            bass_nofuse=True,
                            engine=inst.engine,
                        )
                        new_insts.append(nop)
                        n_split += 1
                    si.on_wait.clear()
                    for w in keep:
                        si.on_wait.append(w)
                new_insts.append(inst)
            bb.instructions.clear()
            for i in new_insts:
                bb.instructions.append(i)
    return n_split
# ---------------------------------------------------------------------------


def build_nc():
    nc = bass.Bass()

    x_ext = nc.declare_dram_parameter("x", [B_LOC, N, EMB], F32, isOutput=False)
    gam_ext = nc.declare_dram_parameter("ln_gamma", [EMB], F32, isOutput=False)
    bet_ext = nc.declare_dram_parameter("ln_beta", [EMB], F32, isOutput=False)
    wqkv_ext = nc.declare_dram_parameter("w_qkv", [EMB, 3 * INNER], F32, isOutput=False)
    wproj_ext = nc.declare_dram_parameter("w_proj", [INNER, EMB], F32, isOutput=False)
    bproj_ext = nc.declare_dram_parameter("b_proj", [EMB], F32, isOutput=False)
    out_ext = nc.declare_dram_parameter("out", [B_LOC, N, EMB], F32, isOutput=True)

    with tile.TileContext(nc) as tc:
        with (
            tc.tile_pool(name="const", bufs=1) as constp,
            tc.tile_pool(name="persist", bufs=1) as persist,
            tc.tile_pool(name="qk", bufs=1) as qkp,
            tc.tile_pool(name="et", bufs=2) as etp,
            tc.tile_pool(name="attn", bufs=2) as attnp,
            tc.tile_pool(name="stage", bufs=3) as stagep,
            tc.tile_pool(name="small", bufs=2) as smallp,
            tc.tile_pool(name="arena", bufs=1) as arena,
            tc.tile_pool(name="outp", bufs=2) as outpool,
            tc.tile_pool(name="stps", bufs=2, space="PSUM") as st_psum,
            tc.tile_pool(name="bankps", bufs=4, space="PSUM") as bank_psum,
        ):
            # ---------------- constants / weights ----------------
            ident_bf = constp.tile([128, 128], BF16, tag="ident_bf")
            make_identity(nc, ident_bf[:, :])
            ident_f32 = constp.tile([128, 128], F32, tag="ident_f32")
            make_identity(nc, ident_f32[:, :])

            eps_sb = constp.tile([128, 1], F32, tag="eps")
            nc.vector.memset(eps_sb[:, :], EPS)

            gam_sb = constp.tile([128, 1], F32, tag="gam")
            bet_sb = constp.tile([128, 1], F32, tag="bet")
            bproj_sb = constp.tile([128, 1], F32, tag="bproj")
            nc.sync.dma_start(gam_sb[:, :], gam_ext[:].rearrange("(e one) -> e one", one=1))
            nc.sync.dma_start(bet_sb[:, :], bet_ext[:].rearrange("(e one) -> e one", one=1))
            nc.sync.dma_start(bproj_sb[:, :], bproj_ext[:].rearrange("(e one) -> e one", one=1))

            # w_qkv: [emb, 3*inner] f32 -> bf16, emb on partitions (matmul lhsT/rhs)
            wqkv_bf = persist.tile([128, 3 * INNER], BF16, tag="wqkv_bf")
            for c in range(6):
                stg = stagep.tile([128, 512], F32, tag="wstage")
                sl = slice(c * 512, (c + 1) * 512)
                nc.sync.dma_start(stg[:, :], wqkv_ext[:, sl])
                nc.vector.tensor_copy(wqkv_bf[:, sl], stg[:, :])

            # w_proj: [(h d), e] -> [d, h, e] bf16
            wproj_bf = persist.tile([128, H, 128], BF16, tag="wproj_bf")
            wproj_r = wproj_ext[:, :].rearrange("(h d) e -> d h e", h=H)
            for c in range(2):
                stg = stagep.tile([128, 4, 128], F32, tag="wpstage")
                hs = slice(c * 4, (c + 1) * 4)
                nc.sync.dma_start(stg[:, :, :], wproj_r[:, hs, :])
                nc.vector.tensor_copy(wproj_bf[:, hs, :], stg[:, :, :])

            # ---------------- input + LayerNorm + transpose ----------------
            # token t = n*128 + p  ->  x_sb[p, n, :]
            x_sb = arena.tile([128, NT, 128], F32, tag="arena_a")
            nc.sync.dma_start(
                x_sb[:, :, :],
                x_ext[:, :, :].rearrange("b (n p) e -> p (b n) e", p=128),
            )

            sum_x = smallp.tile([128, NT], F32, tag="ln_sum")
            mu = smallp.tile([128, NT], F32, tag="ln_mu")
            sumsq = smallp.tile([128, NT], F32, tag="ln_sumsq")
            var = smallp.tile([128, NT], F32, tag="ln_var")
            std = smallp.tile([128, NT], F32, tag="ln_std")
            rstd = smallp.tile([128, NT], F32, tag="ln_rstd")
            nbias = smallp.tile([128, NT], F32, tag="ln_nbias")

            nc.vector.tensor_reduce(
                sum_x[:, :], x_sb[:, :, :], axis=mybir.AxisListType.X, op=ALU.add
            )
            nc.vector.tensor_scalar_mul(mu[:, :], sum_x[:, :], 1.0 / EMB)
            xsq = arena.tile([128, NT, 128], F32, tag="arena_xsq")
            nc.vector.tensor_mul(xsq[:, :, :], x_sb[:, :, :], x_sb[:, :, :])
            nc.vector.tensor_reduce(
                sumsq[:, :], xsq[:, :, :], axis=mybir.AxisListType.X, op=ALU.add
            )
            # var = E[x^2] - mu^2 = sumsq/EMB - mu*mu
            nc.vector.scalar_tensor_tensor(
                out=var[:, :],
                in0=mu[:, :],
                scalar=-1.0,
                in1=mu[:, :],
                op0=ALU.mult,
                op1=ALU.mult,
            )  # var = -mu*mu (temp)
            nc.vector.scalar_tensor_tensor(
                out=var[:, :],
                in0=sumsq[:, :],
                scalar=1.0 / EMB,
                in1=var[:, :],
                op0=ALU.mult,
                op1=ALU.add,
            )  # var = sumsq/EMB - mu*mu
            nc.scalar.activation(std[:, :], var[:, :], AFT.Sqrt, bias=eps_sb[:, :])
            nc.vector.reciprocal(rstd[:, :], std[:, :])
            nc.vector.tensor_mul(nbias[:, :], mu[:, :], rstd[:, :])
            nc.vector.tensor_scalar_mul(nbias[:, :], nbias[:, :], -1.0)

            # normalized token-major tile -> transpose -> xT (gamma/beta in evac)
            xT = persist.tile([128, T], BF16, tag="xT")
            for n in range(NT):
                xn = stagep.tile([128, 128], BF16, tag="xn_bf")
                nc.vector.tensor_scalar(
                    out=xn[:, :],
                    in0=x_sb[:, n, :],
                    scalar1=rstd[:, n : n + 1],
                    scalar2=nbias[:, n : n + 1],
                    op0=ALU.mult,
                    op1=ALU.add,
                )
                tp = bank_psum.tile([128, 512], BF16, tag="bank")
                nc.tensor.transpose(tp[:, 0:128], xn[:, :], ident_bf[:, :])
                nc.vector.tensor_scalar(
                    out=xT[:, n * 128 : (n + 1) * 128],
                    in0=tp[:, 0:128],
                    scalar1=gam_sb[:, :],
                    scalar2=bet_sb[:, :],
                    op0=ALU.mult,
                    op1=ALU.add,
                )

            # ---------------- v for all tokens: [t, (h, d+1)] ----------------
            v_sb = persist.tile([128, NT, H, D + 1], BF16, tag="v_sb")
            nc.vector.memset(v_sb[:, :, :, D : D + 1], 1.0)
            for n in range(NT):
                vp = st_psum.tile([128, 1024], F32, tag="stps")
                for c in range(2):
                    nc.tensor.matmul(
                        vp[:, c * 512 : (c + 1) * 512],
                        xT[:, n * 128 : (n + 1) * 128],
                        wqkv_bf[:, 2 * INNER + c * 512 : 2 * INNER + (c + 1) * 512],
                        start=True,
                        stop=True,
                    )
                nc.vector.tensor_copy(
                    v_sb[:, n, :, 0:D],
                    vp[:, :].rearrange("p (h d) -> p h d", d=D),
                )

            # ---------------- per-batch attention ----------------
            for b in range(B_LOC):
                tb = slice(b * N, (b + 1) * N)

                # qT/kT for this batch: [d, h, i]
                qT = qkp.tile([128, H, N], BF16, tag="qT")
                kT = qkp.tile([128, H, N], BF16, tag="kT")
                for dst, off in ((qT, 0), (kT, INNER)):
                    for h in range(H):
                        qp = st_psum.tile([128, 1024], F32, tag="stps")
                        for c in range(2):
                            nc.tensor.matmul(
                                qp[:, c * 512 : (c + 1) * 512],
                                wqkv_bf[:, off + h * 128 : off + (h + 1) * 128],
                                xT[:, b * N + c * 512 : b * N + (c + 1) * 512],
                                start=True,
                                stop=True,
                            )
                        nc.vector.tensor_copy(dst[:, h, :], qp[:, :])

                attnT_all = arena.tile([128, H, N], BF16, tag="arena_a")

                for h in range(H):
                    # scores^T + exp -> E[j, i] bf16, j on partitions
                    et = etp.tile([128, NT_B, N], BF16, tag="et")
                    for jt in range(NT_B):
                        stp = st_psum.tile([128, 1024], F32, tag="stps")
                        for c in range(2):
                            nc.tensor.matmul(
                                stp[:, c * 512 : (c + 1) * 512],
                                kT[:, h, jt * 128 : (jt + 1) * 128],
                                qT[:, h, c * 512 : (c + 1) * 512],
                                start=True,
                                stop=True,
                            )
                        nc.scalar.activation(
                            et[:, jt, :], stp[:, :], AFT.Exp, scale=SCALE
                        )

                    # PV: E^T @ [v | 1] per 128-token i-chunk; Z rides col 128
                    attn_sb = attnp.tile([128, NT_B, D], BF16, tag="attn_sb")
                    zr = smallp.tile([128, NT_B], F32, tag="zr")
                    for ic in range(NT_B):
                        pv = bank_psum.tile([128, 512], F32, tag="bank")
                        for jt in range(NT_B):
                            nc.tensor.matmul(
                                pv[:, 0 : D + 1],
                                et[:, jt, ic * 128 : (ic + 1) * 128],
                                v_sb[:, b * NT_B + jt, h, :],
                                start=(jt == 0),
                                stop=(jt == NT_B - 1),
                            )
                        nc.vector.reciprocal(zr[:, ic : ic + 1], pv[:, D : D + 1])
                        nc.vector.tensor_scalar_mul(
                            attn_sb[:, ic, :], pv[:, 0:D], zr[:, ic : ic + 1]
                        )

                    # transpose attn -> [d, i] and stash per-head
                    for half in range(2):
                        atp = bank_psum.tile([128, 512], BF16, tag="bank")
                        for q in range(4):
                            ic = half * 4 + q
                            nc.tensor.transpose(
                                atp[:, q * 128 : (q + 1) * 128],
                                attn_sb[:, ic, :],
                                ident_bf[:, :],
                            )
                        nc.vector.tensor_copy(
                            attnT_all[:, h, half * 512 : (half + 1) * 512], atp[:, :]
                        )

                # projection: finalT[e, t] accumulated over heads
                fin_sb = outpool.tile([128, N], F32, tag="fin_sb")
                for half in range(2):
                    fp = bank_psum.tile([128, 512], F32, tag="bank")
                    sl = slice(half * 512, (half + 1) * 512)
                    for h in range(H):
                        nc.tensor.matmul(
                            fp[:, :],
                            wproj_bf[:, h, :],
                            attnT_all[:, h, sl],
                            start=(h == 0),
                            stop=(h == H - 1),
                        )
                    nc.vector.tensor_scalar_add(fin_sb[:, sl], fp[:, :], bproj_sb[:, :])

                # back to token-major + DMA out
                out_sb = outpool.tile([128, NT_B, 128], F32, tag="out_sb")
                for half in range(2):
                    otp = bank_psum.tile([128, 512], F32, tag="bank")
                    for q in range(4):
                        c = half * 4 + q
                        nc.tensor.transpose(
                            otp[:, q * 128 : (q + 1) * 128],
                            fin_sb[:, c * 128 : (c + 1) * 128],
                            ident_f32[:, :],
                        )
                    nc.vector.tensor_copy(
                        out_sb[:, half * 4 : (half + 1) * 4, :], otp[:, :].rearrange("p (c e) -> p c e", e=128)
                    )
                nc.sync.dma_start(
                    out_ext[b, :, :].rearrange("(c p) e -> p c e", p=128),
                    out_sb[:, :, :],
                )

    split_sync_waits(nc, max_waits=1)
    return nc


_CACHED = {}


def _get_nc():
    if "nc" not in _CACHED:
        _CACHED["nc"] = build_nc()
    return _CACHED["nc"]


def run(inputs, trace=False, trace_kwargs=None):
    """inputs: full-problem dict as from setup_inputs(). Returns (out, results)."""
    x = np.ascontiguousarray(np.asarray(inputs["inputs"], dtype=np.float32))
    shared = {
        "ln_gamma": np.ascontiguousarray(np.asarray(inputs["ln_gamma"], np.float32)),
        "ln_beta": np.ascontiguousarray(np.asarray(inputs["ln_beta"], np.float32)),
        "w_qkv": np.ascontiguousarray(np.asarray(inputs["w_qkv"], np.float32)),
        "w_proj": np.ascontiguousarray(np.asarray(inputs["w_proj"], np.float32)),
        "b_proj": np.ascontiguousarray(np.asarray(inputs["b_proj"], np.float32)),
    }
    in_maps = []
    for i in range(N_CORES):
        m = dict(shared)
        m["x"] = np.ascontiguousarray(x[i * B_LOC : (i + 1) * B_LOC])
        in_maps.append(m)

    nc = _get_nc()
    kw = {}
    if trace:
        kw["trace"] = True
        if trace_kwargs:
            kw["trace_kwargs"] = trace_kwargs
    res = run_bass_kernel_spmd(nc, in_maps, list(range(N_CORES)), **kw)
    out = np.concatenate([res.results[i]["out"] for i in range(N_CORES)], axis=0)
    return out, res


def kernel(**inputs) -> np.ndarray:
    out, _ = run(inputs)
    return out


# revision 9
# speedup vs baseline: 1.1069x; 1.1069x over previous
"""Fused LN + multi-head attention block for Trainium2, data-parallel over 8 NeuronCores.

Problem (hardcoded): B=16, N=1024, EMB=128, H=8, INNER=1024, fp32 I/O.
Each core handles 2 batches; no cross-core communication is needed.

Per-core pipeline (all matmuls in bf16, accumulation fp32 in PSUM):
  1. LayerNorm in token-major tiles, TensorE-transpose -> xT[emb, 2048] bf16
     (gamma/beta folded into the transpose-PSUM evacuation).
  2. qT/kT[d, t] per head via w-as-lhsT matmuls; v token-major [t, (h, d+1)]
     with a constant ones column appended per head.
  3. Scores transposed: ST[j, i] = sum_d k[j,d] q[i,d]; exp on ScalarE with
     the 1/sqrt(INNER) scale folded into the activation scale. No
     max-subtraction: scores are ~N(0, 0.35) by construction, exp is safe.
  4. PV with exp(ST) tiles as weights: out[i, (d, Z)] = E^T @ [v | 1]; the
     ones column yields the softmax denominator Z_i in column 128 for free.
     Normalization = per-partition tensor_scalar multiply by 1/Z during the
     PSUM evacuation.
  5. TensorE-transpose attn -> [d, i], project with w_proj chunks as lhsT
     accumulating heads in PSUM, add bias, transpose back to token-major,
     DMA out.
"""

import sys

for _p in ("/opt/trn_rl_repo",):
    if _p not in sys.path:
        sys.path.insert(0, _p)

import numpy as np

import concourse.bass as bass
import concourse.mybir as mybir
import concourse.tile as tile
from concourse.masks import make_identity
from concourse.bass_utils import run_bass_kernel_spmd

F32 = mybir.dt.float32
BF16 = mybir.dt.bfloat16
ALU = mybir.AluOpType
AFT = mybir.ActivationFunctionType

N_CORES = 8
B = 16
N = 1024
EMB = 128
H = 8
D = 128
INNER = EMB * H
B_LOC = B // N_CORES          # 2 batches per core
T = B_LOC * N                 # 2048 tokens per core
NT = T // 128                 # 16 token tiles per core
NT_B = N // 128               # 8 token tiles per batch
SCALE = float(INNER) ** -0.5  # 1/32, folded into exp()
EPS = 1e-5


# ---------------------------------------------------------------------------
# Workaround: this walrus build rejects instructions carrying more than a
# couple of embedded semaphore waits ("Too many sync wait commands"). After
# Tile scheduling, split excess waits onto standalone same-engine NoOps
# placed immediately before the instruction (engine program order preserves
# the blocking semantics).
def split_sync_waits(nc, max_waits=1):
    n_split = 0
    for f in nc.m.functions:
        for bb in f.blocks:
            new_insts = []
            for inst in bb.instructions:
                si = getattr(inst, "sync_info", None)
                waits = list(si.on_wait) if (si is not None and si.on_wait) else []
                if len(waits) > max_waits:
                    keep = waits[:max_waits]
                    extra = waits[max_waits:]
                    for k, w in enumerate(extra):
                        nop = mybir.InstNoOp(
                            name=f"{inst.name}-wsplit{k}",
                            sync_info=mybir.SyncInfo(on_wait=[w], on_update=[]),
                            bass_nofuse=True,
                            engine=inst.engine,
                        )
                        new_insts.append(nop)
                        n_split += 1
                    si.on_wait.clear()
                    for w in keep:
                        si.on_wait.append(w)
                new_insts.append(inst)
            bb.instructions.clear()
            for i in new_insts:
                bb.instructions.append(i)
    return n_split
# ---------------------------------------------------------------------------


def build_nc():
    nc = bass.Bass()

    x_ext = nc.declare_dram_parameter("x", [B_LOC, N, EMB], F32, isOutput=False)
    gam_ext = nc.declare_dram_parameter("ln_gamma", [EMB], F32, isOutput=False)
    bet_ext = nc.declare_dram_parameter("ln_beta", [EMB], F32, isOutput=False)
    wqkv_ext = nc.declare_dram_parameter("w_qkv", [EMB, 3 * INNER], F32, isOutput=False)
    wproj_ext = nc.declare_dram_parameter("w_proj", [INNER, EMB], F32, isOutput=False)
    bproj_ext = nc.declare_dram_parameter("b_proj", [EMB], F32, isOutput=False)
    out_ext = nc.declare_dram_parameter("out", [B_LOC, N, EMB], F32, isOutput=True)

    with tile.TileContext(nc) as tc:
        with (
            tc.tile_pool(name="const", bufs=1) as constp,
            tc.tile_pool(name="persist", bufs=1) as persist,
            tc.tile_pool(name="qk", bufs=1) as qkp,
            tc.tile_pool(name="et", bufs=2) as etp,
            tc.tile_pool(name="attn", bufs=2) as attnp,
            tc.tile_pool(name="stage", bufs=3) as stagep,
            tc.tile_pool(name="small", bufs=2) as smallp,
            tc.tile_pool(name="arena", bufs=1) as arena,
            tc.tile_pool(name="outp", bufs=2) as outpool,
            tc.tile_pool(name="stps", bufs=2, space="PSUM") as st_psum,
            tc.tile_pool(name="bankps", bufs=4, space="PSUM") as bank_psum,
        ):
            # ---------------- constants / weights ----------------
            ident_bf = constp.tile([128, 128], BF16, tag="ident_bf")
            make_identity(nc, ident_bf[:, :])
            ident_f32 = constp.tile([128, 128], F32, tag="ident_f32")
            make_identity(nc, ident_f32[:, :])

            eps_sb = constp.tile([128, 1], F32, tag="eps")
            nc.vector.memset(eps_sb[:, :], EPS)

            gam_sb = constp.tile([128, 1], F32, tag="gam")
            bet_sb = constp.tile([128, 1], F32, tag="bet")
            bproj_sb = constp.tile([128, 1], F32, tag="bproj")
            nc.sync.dma_start(gam_sb[:, :], gam_ext[:].rearrange("(e one) -> e one", one=1))
            nc.sync.dma_start(bet_sb[:, :], bet_ext[:].rearrange("(e one) -> e one", one=1))
            nc.sync.dma_start(bproj_sb[:, :], bproj_ext[:].rearrange("(e one) -> e one", one=1))

            # w_qkv: [emb, 3*inner] f32 -> bf16, emb on partitions (matmul lhsT/rhs)
            wqkv_bf = persist.tile([128, 3 * INNER], BF16, tag="wqkv_bf")
            for c in range(6):
                stg = stagep.tile([128, 512], F32, tag="wstage")
                sl = slice(c * 512, (c + 1) * 512)
                nc.sync.dma_start(stg[:, :], wqkv_ext[:, sl])
                nc.vector.tensor_copy(wqkv_bf[:, sl], stg[:, :])

            # w_proj: [(h d), e] -> [d, h, e] bf16
            wproj_bf = persist.tile([128, H, 128], BF16, tag="wproj_bf")
            wproj_r = wproj_ext[:, :].rearrange("(h d) e -> d h e", h=H)
            for c in range(2):
                stg = stagep.tile([128, 4, 128], F32, tag="wpstage")
                hs = slice(c * 4, (c + 1) * 4)
                nc.sync.dma_start(stg[:, :, :], wproj_r[:, hs, :])
                nc.vector.tensor_copy(wproj_bf[:, hs, :], stg[:, :, :])

            # ---------------- input + LayerNorm + transpose ----------------
            # token t = n*128 + p  ->  x_sb[p, n, :]
            x_sb = arena.tile([128, NT, 128], F32, tag="arena_a")
            nc.sync.dma_start(
                x_sb[:, :, :],
                x_ext[:, :, :].rearrange("b (n p) e -> p (b n) e", p=128),
            )

            sum_x = smallp.tile([128, NT], F32, tag="ln_sum")
            mu = smallp.tile([128, NT], F32, tag="ln_mu")
            sumsq = smallp.tile([128, NT], F32, tag="ln_sumsq")
            var = smallp.tile([128, NT], F32, tag="ln_var")
            std = smallp.tile([128, NT], F32, tag="ln_std")
            rstd = smallp.tile([128, NT], F32, tag="ln_rstd")
            nbias = smallp.tile([128, NT], F32, tag="ln_nbias")

            nc.vector.tensor_reduce(
                sum_x[:, :], x_sb[:, :, :], axis=mybir.AxisListType.X, op=ALU.add
            )
            nc.vector.tensor_scalar_mul(mu[:, :], sum_x[:, :], 1.0 / EMB)
            xsq = arena.tile([128, NT, 128], F32, tag="arena_xsq")
            nc.vector.tensor_mul(xsq[:, :, :], x_sb[:, :, :], x_sb[:, :, :])
            nc.vector.tensor_reduce(
                sumsq[:, :], xsq[:, :, :], axis=mybir.AxisListType.X, op=ALU.add
            )
            # var = E[x^2] - mu^2 = sumsq/EMB - mu*mu
            nc.vector.scalar_tensor_tensor(
                out=var[:, :],
                in0=mu[:, :],
                scalar=-1.0,
                in1=mu[:, :],
                op0=ALU.mult,
                op1=ALU.mult,
            )  # var = -mu*mu (temp)
            nc.vector.scalar_tensor_tensor(
                out=var[:, :],
                in0=sumsq[:, :],
                scalar=1.0 / EMB,
                in1=var[:, :],
                op0=ALU.mult,
                op1=ALU.add,
            )  # var = sumsq/EMB - mu*mu
            nc.scalar.activation(std[:, :], var[:, :], AFT.Sqrt, bias=eps_sb[:, :])
            nc.vector.reciprocal(rstd[:, :], std[:, :])
            nc.vector.tensor_mul(nbias[:, :], mu[:, :], rstd[:, :])
            nc.vector.tensor_scalar_mul(nbias[:, :], nbias[:, :], -1.0)

            # normalized token-major tile -> transpose -> xT (gamma/beta in evac)
            xT = persist.tile([128, T], BF16, tag="xT")
            for n in range(NT):
                xn = stagep.tile([128, 128], BF16, tag="xn_bf")
                nc.vector.tensor_scalar(
                    out=xn[:, :],
                    in0=x_sb[:, n, :],
                    scalar1=rstd[:, n : n + 1],
                    scalar2=nbias[:, n : n + 1],
                    op0=ALU.mult,
                    op1=ALU.add,
                )
                tp = bank_psum.tile([128, 512], BF16, tag="bank")
                nc.tensor.transpose(tp[:, 0:128], xn[:, :], ident_bf[:, :])
                nc.vector.tensor_scalar(
                    out=xT[:, n * 128 : (n + 1) * 128],
                    in0=tp[:, 0:128],
                    scalar1=gam_sb[:, :],
                    scalar2=bet_sb[:, :],
                    op0=ALU.mult,
                    op1=ALU.add,
                )

            # ---------------- v for all tokens: [t, (h, d+1)] ----------------
            v_sb = persist.tile([128, NT, H, D + 1], BF16, tag="v_sb")
            nc.vector.memset(v_sb[:, :, :, D : D + 1], 1.0)
            for n in range(NT):
                vp = st_psum.tile([128, 1024], F32, tag="stps")
                for c in range(2):
                    nc.tensor.matmul(
                        vp[:, c * 512 : (c + 1) * 512],
                        xT[:, n * 128 : (n + 1) * 128],
                        wqkv_bf[:, 2 * INNER + c * 512 : 2 * INNER + (c + 1) * 512],
                        start=True,
                        stop=True,
                    )
                nc.vector.tensor_copy(
                    v_sb[:, n, :, 0:D],
                    vp[:, :].rearrange("p (h d) -> p h d", d=D),
                )

            # ---------------- per-batch attention ----------------
            # Software-pipelined by one head: head (b,h)'s score matmuls are
            # interleaved with head (b,h-1)'s PV/transpose work so ScalarE's
            # exp runs concurrently with TensorE's PV phase.

            def emit_pv_chunk(prev, ic):
                b0, h0, et0, attn0, zr0 = prev
                pv = bank_psum.tile([128, 512], F32, tag="bank")
                for jt in range(NT_B):
                    nc.tensor.matmul(
                        pv[:, 0 : D + 1],
                        et0[:, jt, ic * 128 : (ic + 1) * 128],
                        v_sb[:, b0 * NT_B + jt, h0, :],
                        start=(jt == 0),
                        stop=(jt == NT_B - 1),
                    )
                nc.vector.reciprocal(zr0[:, ic : ic + 1], pv[:, D : D + 1])
                nc.vector.tensor_scalar_mul(
                    attn0[:, ic, :], pv[:, 0:D], zr0[:, ic : ic + 1]
                )

            def emit_transpose_half(prev, attnT_dst, half):
                b0, h0, et0, attn0, zr0 = prev
                atp = bank_psum.tile([128, 512], BF16, tag="bank")
                for q in range(4):
                    ic = half * 4 + q
                    nc.tensor.transpose(
                        atp[:, q * 128 : (q + 1) * 128],
                        attn0[:, ic, :],
                        ident_bf[:, :],
                    )
                nc.vector.tensor_copy(
                    attnT_dst[:, h0, half * 512 : (half + 1) * 512], atp[:, :]
                )

            prev = None
            prev_attnT = None
            for b in range(B_LOC):
                # qT/kT for this batch: [d, h, i]
                qT = qkp.tile([128, H, N], BF16, tag="qT")
                kT = qkp.tile([128, H, N], BF16, tag="kT")
                for dst, off in ((qT, 0), (kT, INNER)):
                    for h in range(H):
                        qp = st_psum.tile([128, 1024], F32, tag="stps")
                        for c in range(2):
                            nc.tensor.matmul(
                                qp[:, c * 512 : (c + 1) * 512],
                                wqkv_bf[:, off + h * 128 : off + (h + 1) * 128],
                                xT[:, b * N + c * 512 : b * N + (c + 1) * 512],
                                start=True,
                                stop=True,
                            )
                        nc.vector.tensor_copy(dst[:, h, :], qp[:, :])

                attnT_all = arena.tile([128, H, N], BF16, tag="arena_a")

                for h in range(H):
                    # scores^T + exp -> E[j, i] bf16 (j on partitions),
                    # interleaved with PV/transposes of the previous head.
                    et = etp.tile([128, NT_B, N], BF16, tag="et")
                    attn_sb = attnp.tile([128, NT_B, D], BF16, tag="attn_sb")
                    zr = smallp.tile([128, NT_B], F32, tag="zr")
                    for jt in range(NT_B):
                        stp = st_psum.tile([128, 1024], F32, tag="stps")
                        for c in range(2):
                            nc.tensor.matmul(
                                stp[:, c * 512 : (c + 1) * 512],
                                kT[:, h, jt * 128 : (jt + 1) * 128],
                                qT[:, h, c * 512 : (c + 1) * 512],
                                start=True,
                                stop=True,
                            )
                        nc.scalar.activation(
                            et[:, jt, :], stp[:, :], AFT.Exp, scale=SCALE
                        )
                        if prev is not None:
                            emit_pv_chunk(prev, jt)
                            if jt == 5:
                                emit_transpose_half(prev, prev_attnT, 0)
                    if prev is not None:
                        emit_transpose_half(prev, prev_attnT, 1)
                    prev = (b, h, et, attn_sb, zr)
                    prev_attnT = attnT_all

                # flush PV of the batch's last head (PE-only tail; overlaps
                # with the next batch's qk matmuls / projection below)
                for ic in range(NT_B):
                    emit_pv_chunk(prev, ic)
                emit_transpose_half(prev, attnT_all, 0)
                emit_transpose_half(prev, attnT_all, 1)
                prev = None

                # projection: finalT[e, t] accumulated over heads
                fin_sb = outpool.tile([128, N], F32, tag="fin_sb")
                for half in range(2):
                    fp = bank_psum.tile([128, 512], F32, tag="bank")
                    sl = slice(half * 512, (half + 1) * 512)
                    for h in range(H):
                        nc.tensor.matmul(
                            fp[:, :],
                            wproj_bf[:, h, :],
                            attnT_all[:, h, sl],
                            start=(h == 0),
                            stop=(h == H - 1),
                        )
                    nc.vector.tensor_scalar_add(fin_sb[:, sl], fp[:, :], bproj_sb[:, :])

                # back to token-major + DMA out
                out_sb = outpool.tile([128, NT_B, 128], F32, tag="out_sb")
                for half in range(2):
                    otp = bank_psum.tile([128, 512], F32, tag="bank")
                    for q in range(4):
                        c = half * 4 + q
                        nc.tensor.transpose(
                            otp[:, q * 128 : (q + 1) * 128],
                            fin_sb[:, c * 128 : (c + 1) * 128],
                            ident_f32[:, :],
                        )
                    nc.vector.tensor_copy(
                        out_sb[:, half * 4 : (half + 1) * 4, :], otp[:, :].rearrange("p (c e) -> p c e", e=128)
                    )
                nc.sync.dma_start(
                    out_ext[b, :, :].rearrange("(c p) e -> p c e", p=128),
                    out_sb[:, :, :],
                )

    split_sync_waits(nc, max_waits=1)
    return nc


_CACHED = {}


def _get_nc():
    if "nc" not in _CACHED:
        _CACHED["nc"] = build_nc()
    return _CACHED["nc"]


def run(inputs, trace=False, trace_kwargs=None):
    """inputs: full-problem dict as from setup_inputs(). Returns (out, results)."""
    x = np.ascontiguousarray(np.asarray(inputs["inputs"], dtype=np.float32))
    shared = {
        "ln_gamma": np.ascontiguousarray(np.asarray(inputs["ln_gamma"], np.float32)),
        "ln_beta": np.ascontiguousarray(np.asarray(inputs["ln_beta"], np.float32)),
        "w_qkv": np.ascontiguousarray(np.asarray(inputs["w_qkv"], np.float32)),
        "w_proj": np.ascontiguousarray(np.asarray(inputs["w_proj"], np.float32)),
        "b_proj": np.ascontiguousarray(np.asarray(inputs["b_proj"], np.float32)),
    }
    in_maps = []
    for i in range(N_CORES):
        m = dict(shared)
        m["x"] = np.ascontiguousarray(x[i * B_LOC : (i + 1) * B_LOC])
        in_maps.append(m)

    nc = _get_nc()
    kw = {}
    if trace:
        kw["trace"] = True
        if trace_kwargs:
            kw["trace_kwargs"] = trace_kwargs
    res = run_bass_kernel_spmd(nc, in_maps, list(range(N_CORES)), **kw)
    out = np.concatenate([res.results[i]["out"] for i in range(N_CORES)], axis=0)
    return out, res


def kernel(**inputs) -> np.ndarray:
    out, _ = run(inputs)
    return out


# revision 11
# speedup vs baseline: 1.1524x; 1.0411x over previous
"""Fused LN + multi-head attention block for Trainium2, data-parallel over 8 NeuronCores.

Problem (hardcoded): B=16, N=1024, EMB=128, H=8, INNER=1024, fp32 I/O.
Each core handles 2 batches; no cross-core communication is needed.

Per-core pipeline (all matmuls in bf16, accumulation fp32 in PSUM):
  1. LayerNorm in token-major tiles, TensorE-transpose -> xT[emb, 2048] bf16
     (gamma/beta folded into the transpose-PSUM evacuation).
  2. qT/kT[d, t] per head via w-as-lhsT matmuls; v token-major [t, (h, d+1)]
     with a constant ones column appended per head.
  3. Scores transposed: ST[j, i] = sum_d k[j,d] q[i,d]; exp on ScalarE with
     the 1/sqrt(INNER) scale folded into the activation scale. No
     max-subtraction: scores are ~N(0, 0.35) by construction, exp is safe.
  4. PV with exp(ST) tiles as weights: out[i, (d, Z)] = E^T @ [v | 1]; the
     ones column yields the softmax denominator Z_i in column 128 for free.
     Normalization = per-partition tensor_scalar multiply by 1/Z during the
     PSUM evacuation.
  5. TensorE-transpose attn -> [d, i], project with w_proj chunks as lhsT
     accumulating heads in PSUM, add bias, transpose back to token-major,
     DMA out.
"""

import sys

for _p in ("/opt/trn_rl_repo",):
    if _p not in sys.path:
        sys.path.insert(0, _p)

import numpy as np

import concourse.bass as bass
import concourse.mybir as mybir
import concourse.tile as tile
from concourse.masks import make_identity
from concourse.bass_utils import run_bass_kernel_spmd

F32 = mybir.dt.float32
BF16 = mybir.dt.bfloat16
ALU = mybir.AluOpType
AFT = mybir.ActivationFunctionType

N_CORES = 8
B = 16
N = 1024
EMB = 128
H = 8
D = 128
INNER = EMB * H
B_LOC = B // N_CORES          # 2 batches per core
T = B_LOC * N                 # 2048 tokens per core
NT = T // 128                 # 16 token tiles per core
NT_B = N // 128               # 8 token tiles per batch
SCALE = float(INNER) ** -0.5  # 1/32, folded into exp()
EPS = 1e-5


# ---------------------------------------------------------------------------
# Workaround: this walrus build rejects instructions carrying more than a
# couple of embedded semaphore waits ("Too many sync wait commands"). After
# Tile scheduling, split excess waits onto standalone same-engine NoOps
# placed immediately before the instruction (engine program order preserves
# the blocking semantics).
def split_sync_waits(nc, max_waits=1):
    n_split = 0
    for f in nc.m.functions:
        for bb in f.blocks:
            new_insts = []
            for inst in bb.instructions:
                si = getattr(inst, "sync_info", None)
                waits = list(si.on_wait) if (si is not None and si.on_wait) else []
                if len(waits) > max_waits:
                    keep = waits[:max_waits]
                    extra = waits[max_waits:]
                    for k, w in enumerate(extra):
                        nop = mybir.InstNoOp(
                            name=f"{inst.name}-wsplit{k}",
                            sync_info=mybir.SyncInfo(on_wait=[w], on_update=[]),
                            bass_nofuse=True,
                            engine=inst.engine,
                        )
                        new_insts.append(nop)
                        n_split += 1
                    si.on_wait.clear()
                    for w in keep:
                        si.on_wait.append(w)
                new_insts.append(inst)
            bb.instructions.clear()
            for i in new_insts:
                bb.instructions.append(i)
    return n_split
# ---------------------------------------------------------------------------


def build_nc():
    nc = bass.Bass()

    x_ext = nc.declare_dram_parameter("x", [B_LOC, N, EMB], F32, isOutput=False)
    gam_ext = nc.declare_dram_parameter("ln_gamma", [EMB], F32, isOutput=False)
    bet_ext = nc.declare_dram_parameter("ln_beta", [EMB], F32, isOutput=False)
    wqkv_ext = nc.declare_dram_parameter("w_qkv", [EMB, 3 * INNER], F32, isOutput=False)
    wproj_ext = nc.declare_dram_parameter("w_proj", [INNER, EMB], F32, isOutput=False)
    bproj_ext = nc.declare_dram_parameter("b_proj", [EMB], F32, isOutput=False)
    out_ext = nc.declare_dram_parameter("out", [B_LOC, N, EMB], F32, isOutput=True)

    with tile.TileContext(nc) as tc:
        with (
            tc.tile_pool(name="const", bufs=1) as constp,
            tc.tile_pool(name="persist", bufs=1) as persist,
            tc.tile_pool(name="qk", bufs=1) as qkp,
            tc.tile_pool(name="et", bufs=2) as etp,
            tc.tile_pool(name="attn", bufs=2) as attnp,
            tc.tile_pool(name="stage", bufs=3) as stagep,
            tc.tile_pool(name="small", bufs=2) as smallp,
            tc.tile_pool(name="arena", bufs=1) as arena,
            tc.tile_pool(name="outp", bufs=2) as outpool,
            tc.tile_pool(name="stps", bufs=2, space="PSUM") as st_psum,
            tc.tile_pool(name="bankps", bufs=4, space="PSUM") as bank_psum,
        ):
            # ---------------- constants / weights ----------------
            ident_bf = constp.tile([128, 128], BF16, tag="ident_bf")
            make_identity(nc, ident_bf[:, :])
            ident_f32 = constp.tile([128, 128], F32, tag="ident_f32")
            make_identity(nc, ident_f32[:, :])

            eps_sb = constp.tile([128, 1], F32, tag="eps")
            nc.vector.memset(eps_sb[:, :], EPS)

            gam_sb = constp.tile([128, 1], F32, tag="gam")
            bet_sb = constp.tile([128, 1], F32, tag="bet")
            bproj_sb = constp.tile([128, 1], F32, tag="bproj")
            nc.sync.dma_start(gam_sb[:, :], gam_ext[:].rearrange("(e one) -> e one", one=1))
            nc.sync.dma_start(bet_sb[:, :], bet_ext[:].rearrange("(e one) -> e one", one=1))
            nc.sync.dma_start(bproj_sb[:, :], bproj_ext[:].rearrange("(e one) -> e one", one=1))

            # w_qkv: [emb, 3*inner] f32 -> bf16, emb on partitions (matmul lhsT/rhs)
            wqkv_bf = persist.tile([128, 3 * INNER], BF16, tag="wqkv_bf")
            for c in range(6):
                stg = stagep.tile([128, 512], F32, tag="wstage")
                sl = slice(c * 512, (c + 1) * 512)
                nc.sync.dma_start(stg[:, :], wqkv_ext[:, sl])
                nc.vector.tensor_copy(wqkv_bf[:, sl], stg[:, :])

            # w_proj: [(h d), e] -> [d, h, e] bf16
            wproj_bf = persist.tile([128, H, 128], BF16, tag="wproj_bf")
            wproj_r = wproj_ext[:, :].rearrange("(h d) e -> d h e", h=H)
            for c in range(2):
                stg = stagep.tile([128, 4, 128], F32, tag="wpstage")
                hs = slice(c * 4, (c + 1) * 4)
                nc.sync.dma_start(stg[:, :, :], wproj_r[:, hs, :])
                nc.vector.tensor_copy(wproj_bf[:, hs, :], stg[:, :, :])

            # ---------------- input + LayerNorm + transpose ----------------
            # token t = n*128 + p  ->  x_sb[p, n, :]
            x_sb = arena.tile([128, NT, 128], F32, tag="arena_a")
            nc.sync.dma_start(
                x_sb[:, :, :],
                x_ext[:, :, :].rearrange("b (n p) e -> p (b n) e", p=128),
            )

            sum_x = smallp.tile([128, NT], F32, tag="ln_sum")
            mu = smallp.tile([128, NT], F32, tag="ln_mu")
            sumsq = smallp.tile([128, NT], F32, tag="ln_sumsq")
            var = smallp.tile([128, NT], F32, tag="ln_var")
            std = smallp.tile([128, NT], F32, tag="ln_std")
            rstd = smallp.tile([128, NT], F32, tag="ln_rstd")
            nbias = smallp.tile([128, NT], F32, tag="ln_nbias")

            nc.vector.tensor_reduce(
                sum_x[:, :], x_sb[:, :, :], axis=mybir.AxisListType.X, op=ALU.add
            )
            nc.vector.tensor_scalar_mul(mu[:, :], sum_x[:, :], 1.0 / EMB)
            xsq = arena.tile([128, NT, 128], F32, tag="arena_xsq")
            nc.vector.tensor_mul(xsq[:, :, :], x_sb[:, :, :], x_sb[:, :, :])
            nc.vector.tensor_reduce(
                sumsq[:, :], xsq[:, :, :], axis=mybir.AxisListType.X, op=ALU.add
            )
            # var = E[x^2] - mu^2 = sumsq/EMB - mu*mu
            nc.vector.scalar_tensor_tensor(
                out=var[:, :],
                in0=mu[:, :],
                scalar=-1.0,
                in1=mu[:, :],
                op0=ALU.mult,
                op1=ALU.mult,
            )  # var = -mu*mu (temp)
            nc.vector.scalar_tensor_tensor(
                out=var[:, :],
                in0=sumsq[:, :],
                scalar=1.0 / EMB,
                in1=var[:, :],
                op0=ALU.mult,
                op1=ALU.add,
            )  # var = sumsq/EMB - mu*mu
            nc.scalar.activation(std[:, :], var[:, :], AFT.Sqrt, bias=eps_sb[:, :])
            nc.vector.reciprocal(rstd[:, :], std[:, :])
            nc.vector.tensor_mul(nbias[:, :], mu[:, :], rstd[:, :])
            nc.vector.tensor_scalar_mul(nbias[:, :], nbias[:, :], -1.0)

            # normalized token-major tile -> transpose -> xT (gamma/beta in evac)
            xT = persist.tile([128, T], BF16, tag="xT")
            for g in range(NT // 4):
                tp = bank_psum.tile([128, 4, 128], BF16, tag="bank")
                for q in range(4):
                    n = g * 4 + q
                    xn = stagep.tile([128, 128], BF16, tag="xn_bf")
                    nc.vector.tensor_scalar(
                        out=xn[:, :],
                        in0=x_sb[:, n, :],
                        scalar1=rstd[:, n : n + 1],
                        scalar2=nbias[:, n : n + 1],
                        op0=ALU.mult,
                        op1=ALU.add,
                    )
                    nc.tensor.transpose(tp[:, q, :], xn[:, :], ident_bf[:, :])
                nc.vector.tensor_scalar(
                    out=xT[:, g * 512 : (g + 1) * 512],
                    in0=tp[:, :, :],
                    scalar1=gam_sb[:, :],
                    scalar2=bet_sb[:, :],
                    op0=ALU.mult,
                    op1=ALU.add,
                )

            # ---------------- per-batch attention ----------------
            # Software-pipelined by one head: head (b,h)'s score matmuls are
            # interleaved with head (b,h-1)'s PV/transpose work so ScalarE's
            # exp runs concurrently with TensorE's PV phase.

            def emit_pv_chunk(prev, ic):
                b0, h0, et0, attn0, zr0 = prev
                pv = bank_psum.tile([128, 512], F32, tag="bank")
                for jt in range(NT_B):
                    nc.tensor.matmul(
                        pv[:, 0 : D + 1],
                        et0[:, jt, ic * 128 : (ic + 1) * 128],
                        v_sb[:, b0 * NT_B + jt, h0, :],
                        start=(jt == 0),
                        stop=(jt == NT_B - 1),
                    )
                nc.vector.reciprocal(zr0[:, ic : ic + 1], pv[:, D : D + 1])
                nc.vector.tensor_scalar_mul(
                    attn0[:, ic, :], pv[:, 0:D], zr0[:, ic : ic + 1]
                )

            def emit_transpose_half(prev, attnT_dst, half):
                b0, h0, et0, attn0, zr0 = prev
                atp = bank_psum.tile([128, 512], BF16, tag="bank")
                for q in range(4):
                    ic = half * 4 + q
                    nc.tensor.transpose(
                        atp[:, q * 128 : (q + 1) * 128],
                        attn0[:, ic, :],
                        ident_bf[:, :],
                    )
                nc.vector.tensor_copy(
                    attnT_dst[:, h0, half * 512 : (half + 1) * 512], atp[:, :]
                )

            # alternate PSUM-evacuation copies between DVE and ScalarE (both
            # have slack during the matmul-heavy qkv phases)
            evac_state = {"i": 0}

            def evac_copy(out_ap, in_ap):
                evac_state["i"] += 1
                if evac_state["i"] % 2 == 0:
                    nc.vector.tensor_copy(out_ap, in_ap)
                else:
                    nc.scalar.copy(out_ap, in_ap)

            v_sb = persist.tile([128, NT, H, D + 1], BF16, tag="v_sb")
            nc.vector.memset(v_sb[:, :, :, D : D + 1], 1.0)

            prev = None
            prev_attnT = None
            for b in range(B_LOC):
                # qT/kT for this batch: [d, h, i]
                qT = qkp.tile([128, H, N], BF16, tag="qT")
                kT = qkp.tile([128, H, N], BF16, tag="kT")
                for dst, off in ((qT, 0), (kT, INNER)):
                    for h in range(H):
                        qp = st_psum.tile([128, 1024], F32, tag="stps")
                        for c in range(2):
                            nc.tensor.matmul(
                                qp[:, c * 512 : (c + 1) * 512],
                                wqkv_bf[:, off + h * 128 : off + (h + 1) * 128],
                                xT[:, b * N + c * 512 : b * N + (c + 1) * 512],
                                start=True,
                                stop=True,
                            )
                        evac_copy(dst[:, h, :], qp[:, :])

                if b == 0:
                    # v for all tokens: [t, (h, d+1)]; overlaps the first
                    # heads' score phase on the PE.
                    for n in range(NT):
                        vp = st_psum.tile([128, 1024], F32, tag="stps")
                        for c in range(2):
                            nc.tensor.matmul(
                                vp[:, c * 512 : (c + 1) * 512],
                                xT[:, n * 128 : (n + 1) * 128],
                                wqkv_bf[:, 2 * INNER + c * 512 : 2 * INNER + (c + 1) * 512],
                                start=True,
                                stop=True,
                            )
                        evac_copy(
                            v_sb[:, n, :, 0:D],
                            vp[:, :].rearrange("p (h d) -> p h d", d=D),
                        )

                attnT_all = arena.tile([128, H, N], BF16, tag="arena_a")

                for h in range(H):
                    # scores^T + exp -> E[j, i] bf16 (j on partitions),
                    # interleaved with PV/transposes of the previous head.
                    et = etp.tile([128, NT_B, N], BF16, tag="et")
                    attn_sb = attnp.tile([128, NT_B, D], BF16, tag="attn_sb")
                    zr = smallp.tile([128, NT_B], F32, tag="zr")
                    for jt in range(NT_B):
                        stp = st_psum.tile([128, 1024], F32, tag="stps")
                        for c in range(2):
                            nc.tensor.matmul(
                                stp[:, c * 512 : (c + 1) * 512],
                                kT[:, h, jt * 128 : (jt + 1) * 128],
                                qT[:, h, c * 512 : (c + 1) * 512],
                                start=True,
                                stop=True,
                            )
                        nc.scalar.activation(
                            et[:, jt, :], stp[:, :], AFT.Exp, scale=SCALE
                        )
                        if prev is not None:
                            emit_pv_chunk(prev, jt)
                            if jt == 5:
                                emit_transpose_half(prev, prev_attnT, 0)
                    if prev is not None:
                        emit_transpose_half(prev, prev_attnT, 1)
                    prev = (b, h, et, attn_sb, zr)
                    prev_attnT = attnT_all

                # flush PV of the batch's last head (PE-only tail; overlaps
                # with the next batch's qk matmuls / projection below)
                for ic in range(NT_B):
                    emit_pv_chunk(prev, ic)
                emit_transpose_half(prev, attnT_all, 0)
                emit_transpose_half(prev, attnT_all, 1)
                prev = None

                # projection: finalT[e, t] accumulated over heads
                fin_sb = outpool.tile([128, N], F32, tag="fin_sb")
                for half in range(2):
                    fp = bank_psum.tile([128, 512], F32, tag="bank")
                    sl = slice(half * 512, (half + 1) * 512)
                    for h in range(H):
                        nc.tensor.matmul(
                            fp[:, :],
                            wproj_bf[:, h, :],
                            attnT_all[:, h, sl],
                            start=(h == 0),
                            stop=(h == H - 1),
                        )
                    nc.vector.tensor_scalar_add(fin_sb[:, sl], fp[:, :], bproj_sb[:, :])

                # back to token-major + DMA out
                out_sb = outpool.tile([128, NT_B, 128], F32, tag="out_sb")
                for half in range(2):
                    otp = bank_psum.tile([128, 512], F32, tag="bank")
                    for q in range(4):
                        c = half * 4 + q
                        nc.tensor.transpose(
                            otp[:, q * 128 : (q + 1) * 128],
                            fin_sb[:, c * 128 : (c + 1) * 128],
                            ident_f32[:, :],
                        )
                    nc.vector.tensor_copy(
                        out_sb[:, half * 4 : (half + 1) * 4, :], otp[:, :].rearrange("p (c e) -> p c e", e=128)
                    )
                nc.sync.dma_start(
                    out_ext[b, :, :].rearrange("(c p) e -> p c e", p=128),
                    out_sb[:, :, :],
                )

    split_sync_waits(nc, max_waits=1)
    return nc


_CACHED = {}


def _get_nc():
    if "nc" not in _CACHED:
        _CACHED["nc"] = build_nc()
    return _CACHED["nc"]


def run(inputs, trace=False, trace_kwargs=None):
    """inputs: full-problem dict as from setup_inputs(). Returns (out, results)."""
    x = np.ascontiguousarray(np.asarray(inputs["inputs"], dtype=np.float32))
    shared = {
        "ln_gamma": np.ascontiguousarray(np.asarray(inputs["ln_gamma"], np.float32)),
        "ln_beta": np.ascontiguousarray(np.asarray(inputs["ln_beta"], np.float32)),
        "w_qkv": np.ascontiguousarray(np.asarray(inputs["w_qkv"], np.float32)),
        "w_proj": np.ascontiguousarray(np.asarray(inputs["w_proj"], np.float32)),
        "b_proj": np.ascontiguousarray(np.asarray(inputs["b_proj"], np.float32)),
    }
    in_maps = []
    for i in range(N_CORES):
        m = dict(shared)
        m["x"] = np.ascontiguousarray(x[i * B_LOC : (i + 1) * B_LOC])
        in_maps.append(m)

    nc = _get_nc()
    kw = {}
    if trace:
        kw["trace"] = True
        if trace_kwargs:
            kw["trace_kwargs"] = trace_kwargs
    res = run_bass_kernel_spmd(nc, in_maps, list(range(N_CORES)), **kw)
    out = np.concatenate([res.results[i]["out"] for i in range(N_CORES)], axis=0)
    return out, res


def kernel(**inputs) -> np.ndarray:
    out, _ = run(inputs)
    return out


# revision 12
# speedup vs baseline: 1.2130x; 1.0526x over previous
"""Fused LN + multi-head attention block for Trainium2, data-parallel over 8 NeuronCores.

Problem (hardcoded): B=16, N=1024, EMB=128, H=8, INNER=1024, fp32 I/O.
Each core handles 2 batches; no cross-core communication is needed.

Per-core pipeline (all matmuls in bf16, accumulation fp32 in PSUM):
  1. LayerNorm in token-major tiles, TensorE-transpose -> xT[emb, 2048] bf16
     (gamma/beta folded into the transpose-PSUM evacuation).
  2. qT/kT[d, t] per head via w-as-lhsT matmuls; v token-major [t, (h, d+1)]
     with a constant ones column appended per head.
  3. Scores transposed: ST[j, i] = sum_d k[j,d] q[i,d]; exp on ScalarE with
     the 1/sqrt(INNER) scale folded into the activation scale. No
     max-subtraction: scores are ~N(0, 0.35) by construction, exp is safe.
  4. PV with exp(ST) tiles as weights: out[i, (d, Z)] = E^T @ [v | 1]; the
     ones column yields the softmax denominator Z_i in column 128 for free.
     Normalization = per-partition tensor_scalar multiply by 1/Z during the
     PSUM evacuation.
  5. TensorE-transpose attn -> [d, i], project with w_proj chunks as lhsT
     accumulating heads in PSUM, add bias, transpose back to token-major,
     DMA out.
"""

import sys

for _p in ("/opt/trn_rl_repo",):
    if _p not in sys.path:
        sys.path.insert(0, _p)

import numpy as np

import concourse.bass as bass
import concourse.mybir as mybir
import concourse.tile as tile
from concourse.masks import make_identity
from concourse.bass_utils import run_bass_kernel_spmd

F32 = mybir.dt.float32
BF16 = mybir.dt.bfloat16
ALU = mybir.AluOpType
AFT = mybir.ActivationFunctionType

N_CORES = 8
B = 16
N = 1024
EMB = 128
H = 8
D = 128
INNER = EMB * H
B_LOC = B // N_CORES          # 2 batches per core
T = B_LOC * N                 # 2048 tokens per core
NT = T // 128                 # 16 token tiles per core
NT_B = N // 128               # 8 token tiles per batch
SCALE = float(INNER) ** -0.5  # 1/32, folded into exp()
EPS = 1e-5


# ---------------------------------------------------------------------------
# Workaround: this walrus build rejects instructions carrying more than a
# couple of embedded semaphore waits ("Too many sync wait commands"). After
# Tile scheduling, split excess waits onto standalone same-engine NoOps
# placed immediately before the instruction (engine program order preserves
# the blocking semantics).
def split_sync_waits(nc, max_waits=1):
    n_split = 0
    for f in nc.m.functions:
        for bb in f.blocks:
            new_insts = []
            for inst in bb.instructions:
                si = getattr(inst, "sync_info", None)
                waits = list(si.on_wait) if (si is not None and si.on_wait) else []
                if len(waits) > max_waits:
                    keep = waits[:max_waits]
                    extra = waits[max_waits:]
                    for k, w in enumerate(extra):
                        nop = mybir.InstNoOp(
                            name=f"{inst.name}-wsplit{k}",
                            sync_info=mybir.SyncInfo(on_wait=[w], on_update=[]),
                            bass_nofuse=True,
                            engine=inst.engine,
                        )
                        new_insts.append(nop)
                        n_split += 1
                    si.on_wait.clear()
                    for w in keep:
                        si.on_wait.append(w)
                new_insts.append(inst)
            bb.instructions.clear()
            for i in new_insts:
                bb.instructions.append(i)
    return n_split
# ---------------------------------------------------------------------------


def build_nc():
    nc = bass.Bass()

    x_ext = nc.declare_dram_parameter("x", [B_LOC, N, EMB], F32, isOutput=False)
    gam_ext = nc.declare_dram_parameter("ln_gamma", [EMB], F32, isOutput=False)
    bet_ext = nc.declare_dram_parameter("ln_beta", [EMB], F32, isOutput=False)
    wqkv_ext = nc.declare_dram_parameter("w_qkv", [EMB, 3 * INNER], F32, isOutput=False)
    wproj_ext = nc.declare_dram_parameter("w_proj", [INNER, EMB], F32, isOutput=False)
    bproj_ext = nc.declare_dram_parameter("b_proj", [EMB], F32, isOutput=False)
    out_ext = nc.declare_dram_parameter("out", [B_LOC, N, EMB], F32, isOutput=True)

    with tile.TileContext(nc) as tc:
        with (
            tc.tile_pool(name="const", bufs=1) as constp,
            tc.tile_pool(name="persist", bufs=1) as persist,
            tc.tile_pool(name="qk", bufs=1) as qkp,
            tc.tile_pool(name="et", bufs=2) as etp,
            tc.tile_pool(name="attn", bufs=2) as attnp,
            tc.tile_pool(name="stage", bufs=3) as stagep,
            tc.tile_pool(name="small", bufs=2) as smallp,
            tc.tile_pool(name="arena", bufs=1) as arena,
            tc.tile_pool(name="outp", bufs=2) as outpool,
            tc.tile_pool(name="stps", bufs=2, space="PSUM") as st_psum,
            tc.tile_pool(name="bankps", bufs=4, space="PSUM") as bank_psum,
        ):
            # ---------------- constants / weights ----------------
            ident_bf = constp.tile([128, 128], BF16, tag="ident_bf")
            make_identity(nc, ident_bf[:, :])
            ident_f32 = constp.tile([128, 128], F32, tag="ident_f32")
            make_identity(nc, ident_f32[:, :])

            eps_sb = constp.tile([128, 1], F32, tag="eps")
            nc.vector.memset(eps_sb[:, :], EPS)

            gam_sb = constp.tile([128, 1], F32, tag="gam")
            bet_sb = constp.tile([128, 1], F32, tag="bet")
            bproj_sb = constp.tile([128, 1], F32, tag="bproj")
            nc.sync.dma_start(gam_sb[:, :], gam_ext[:].rearrange("(e one) -> e one", one=1))
            nc.sync.dma_start(bet_sb[:, :], bet_ext[:].rearrange("(e one) -> e one", one=1))
            nc.sync.dma_start(bproj_sb[:, :], bproj_ext[:].rearrange("(e one) -> e one", one=1))

            # w_qkv: [emb, 3*inner] f32 -> bf16, emb on partitions (matmul lhsT/rhs)
            wqkv_bf = persist.tile([128, 3 * INNER], BF16, tag="wqkv_bf")
            for c in range(6):
                stg = stagep.tile([128, 512], F32, tag="wstage")
                sl = slice(c * 512, (c + 1) * 512)
                nc.sync.dma_start(stg[:, :], wqkv_ext[:, sl])
                nc.vector.tensor_copy(wqkv_bf[:, sl], stg[:, :])

            # w_proj: [(h d), e] -> [d, h, e] bf16
            wproj_bf = persist.tile([128, H, 128], BF16, tag="wproj_bf")
            wproj_r = wproj_ext[:, :].rearrange("(h d) e -> d h e", h=H)
            for c in range(2):
                stg = stagep.tile([128, 4, 128], F32, tag="wpstage")
                hs = slice(c * 4, (c + 1) * 4)
                nc.sync.dma_start(stg[:, :, :], wproj_r[:, hs, :])
                nc.vector.tensor_copy(wproj_bf[:, hs, :], stg[:, :, :])

            # ---------------- input + LayerNorm + transpose ----------------
            # token t = n*128 + p  ->  x_sb[p, n, :]
            x_sb = arena.tile([128, NT, 128], F32, tag="arena_a")
            nc.sync.dma_start(
                x_sb[:, :, :],
                x_ext[:, :, :].rearrange("b (n p) e -> p (b n) e", p=128),
            )

            sum_x = smallp.tile([128, NT], F32, tag="ln_sum")
            mu = smallp.tile([128, NT], F32, tag="ln_mu")
            sumsq = smallp.tile([128, NT], F32, tag="ln_sumsq")
            var = smallp.tile([128, NT], F32, tag="ln_var")
            std = smallp.tile([128, NT], F32, tag="ln_std")
            rstd = smallp.tile([128, NT], F32, tag="ln_rstd")
            nbias = smallp.tile([128, NT], F32, tag="ln_nbias")

            nc.vector.tensor_reduce(
                sum_x[:, :], x_sb[:, :, :], axis=mybir.AxisListType.X, op=ALU.add
            )
            nc.vector.tensor_scalar_mul(mu[:, :], sum_x[:, :], 1.0 / EMB)
            xsq = arena.tile([128, NT, 128], F32, tag="arena_xsq")
            nc.vector.tensor_mul(xsq[:, :, :], x_sb[:, :, :], x_sb[:, :, :])
            nc.vector.tensor_reduce(
                sumsq[:, :], xsq[:, :, :], axis=mybir.AxisListType.X, op=ALU.add
            )
            # var = E[x^2] - mu^2 = sumsq/EMB - mu*mu
            nc.vector.scalar_tensor_tensor(
                out=var[:, :],
                in0=mu[:, :],
                scalar=-1.0,
                in1=mu[:, :],
                op0=ALU.mult,
                op1=ALU.mult,
            )  # var = -mu*mu (temp)
            nc.vector.scalar_tensor_tensor(
                out=var[:, :],
                in0=sumsq[:, :],
                scalar=1.0 / EMB,
                in1=var[:, :],
                op0=ALU.mult,
                op1=ALU.add,
            )  # var = sumsq/EMB - mu*mu
            nc.scalar.activation(std[:, :], var[:, :], AFT.Sqrt, bias=eps_sb[:, :])
            nc.vector.reciprocal(rstd[:, :], std[:, :])
            nc.vector.tensor_mul(nbias[:, :], mu[:, :], rstd[:, :])
            nc.vector.tensor_scalar_mul(nbias[:, :], nbias[:, :], -1.0)

            # normalized token-major tile -> transpose -> xT (gamma/beta in evac)
            xT = persist.tile([128, T], BF16, tag="xT")
            for g in range(NT // 4):
                tp = bank_psum.tile([128, 4, 128], BF16, tag="bank")
                for q in range(4):
                    n = g * 4 + q
                    xn = stagep.tile([128, 128], BF16, tag="xn_bf")
                    nc.vector.tensor_scalar(
                        out=xn[:, :],
                        in0=x_sb[:, n, :],
                        scalar1=rstd[:, n : n + 1],
                        scalar2=nbias[:, n : n + 1],
                        op0=ALU.mult,
                        op1=ALU.add,
                    )
                    nc.tensor.transpose(tp[:, q, :], xn[:, :], ident_bf[:, :])
                nc.vector.tensor_scalar(
                    out=xT[:, g * 512 : (g + 1) * 512],
                    in0=tp[:, :, :],
                    scalar1=gam_sb[:, :],
                    scalar2=bet_sb[:, :],
                    op0=ALU.mult,
                    op1=ALU.add,
                )

            # ---------------- per-batch attention ----------------
            # Software-pipelined by one head: head (b,h)'s score matmuls are
            # interleaved with head (b,h-1)'s PV/transpose work so ScalarE's
            # exp runs concurrently with TensorE's PV phase.

            def emit_pv_chunk(prev, ic):
                b0, h0, et0, attn0, zr0 = prev
                pv = bank_psum.tile([128, 512], F32, tag="bank")
                for jt in range(NT_B):
                    nc.tensor.matmul(
                        pv[:, 0 : D + 1],
                        et0[:, jt, ic * 128 : (ic + 1) * 128],
                        v_sb[:, b0 * NT_B + jt, h0, :],
                        start=(jt == 0),
                        stop=(jt == NT_B - 1),
                    )
                nc.vector.reciprocal(zr0[:, ic : ic + 1], pv[:, D : D + 1])
                nc.vector.tensor_scalar_mul(
                    attn0[:, ic, :], pv[:, 0:D], zr0[:, ic : ic + 1]
                )

            def emit_transpose_half(prev, attnT_dst, half):
                b0, h0, et0, attn0, zr0 = prev
                atp = bank_psum.tile([128, 512], BF16, tag="bank")
                for q in range(4):
                    ic = half * 4 + q
                    nc.tensor.transpose(
                        atp[:, q * 128 : (q + 1) * 128],
                        attn0[:, ic, :],
                        ident_bf[:, :],
                    )
                nc.vector.tensor_copy(
                    attnT_dst[:, h0, half * 512 : (half + 1) * 512], atp[:, :]
                )

            # alternate PSUM-evacuation copies between DVE and ScalarE (both
            # have slack during the matmul-heavy qkv phases)
            evac_state = {"i": 0}

            def evac_copy(out_ap, in_ap):
                evac_state["i"] += 1
                if evac_state["i"] % 2 == 0:
                    nc.vector.tensor_copy(out_ap, in_ap)
                else:
                    nc.scalar.copy(out_ap, in_ap)

            v_sb = persist.tile([128, NT, H, D + 1], BF16, tag="v_sb")
            nc.vector.memset(v_sb[:, :, :, D : D + 1], 1.0)

            def emit_v_tile(n):
                for c in range(2):
                    vp = bank_psum.tile([128, 512], F32, tag="bank")
                    nc.tensor.matmul(
                        vp[:, :],
                        xT[:, n * 128 : (n + 1) * 128],
                        wqkv_bf[:, 2 * INNER + c * 512 : 2 * INNER + (c + 1) * 512],
                        start=True,
                        stop=True,
                    )
                    evac_copy(
                        v_sb[:, n, 4 * c : 4 * (c + 1), 0:D],
                        vp[:, :].rearrange("p (h d) -> p h d", d=D),
                    )

            prev = None
            prev_attnT = None
            for b in range(B_LOC):
                if b == 0:
                    # v for batch-0 tokens first: PV(h0) needs all of them
                    for n in range(NT_B):
                        emit_v_tile(n)

                # qT/kT for this batch, interleaved per head so head h's
                # score matmuls can start as soon as its own q/k landed
                qT = qkp.tile([128, H, N], BF16, tag="qT")
                kT = qkp.tile([128, H, N], BF16, tag="kT")
                for h in range(H):
                    for dst, off in ((qT, 0), (kT, INNER)):
                        for c in range(2):
                            qp = bank_psum.tile([128, 512], F32, tag="bank")
                            nc.tensor.matmul(
                                qp[:, :],
                                wqkv_bf[:, off + h * 128 : off + (h + 1) * 128],
                                xT[:, b * N + c * 512 : b * N + (c + 1) * 512],
                                start=True,
                                stop=True,
                            )
                            evac_copy(dst[:, h, c * 512 : (c + 1) * 512], qp[:, :])
                    if b == 0 and h < 4:
                        # batch-1 v tiles, spread across the early heads
                        emit_v_tile(NT_B + 2 * h)
                        emit_v_tile(NT_B + 2 * h + 1)

                attnT_all = arena.tile([128, H, N], BF16, tag="arena_a")

                for h in range(H):
                    # scores^T + exp -> E[j, i] bf16 (j on partitions),
                    # interleaved with PV/transposes of the previous head.
                    et = etp.tile([128, NT_B, N], BF16, tag="et")
                    attn_sb = attnp.tile([128, NT_B, D], BF16, tag="attn_sb")
                    zr = smallp.tile([128, NT_B], F32, tag="zr")
                    for jt in range(NT_B):
                        stp = st_psum.tile([128, 1024], F32, tag="stps")
                        for c in range(2):
                            nc.tensor.matmul(
                                stp[:, c * 512 : (c + 1) * 512],
                                kT[:, h, jt * 128 : (jt + 1) * 128],
                                qT[:, h, c * 512 : (c + 1) * 512],
                                start=True,
                                stop=True,
                            )
                        nc.scalar.activation(
                            et[:, jt, :], stp[:, :], AFT.Exp, scale=SCALE
                        )
                        if prev is not None:
                            emit_pv_chunk(prev, jt)
                            if jt == 5:
                                emit_transpose_half(prev, prev_attnT, 0)
                    if prev is not None:
                        emit_transpose_half(prev, prev_attnT, 1)
                    prev = (b, h, et, attn_sb, zr)
                    prev_attnT = attnT_all

                # flush PV of the batch's last head (PE-only tail; overlaps
                # with the next batch's qk matmuls / projection below)
                for ic in range(NT_B):
                    emit_pv_chunk(prev, ic)
                emit_transpose_half(prev, attnT_all, 0)
                emit_transpose_half(prev, attnT_all, 1)
                prev = None

                # projection: finalT[e, t] accumulated over heads
                fin_sb = outpool.tile([128, N], F32, tag="fin_sb")
                for half in range(2):
                    fp = bank_psum.tile([128, 512], F32, tag="bank")
                    sl = slice(half * 512, (half + 1) * 512)
                    for h in range(H):
                        nc.tensor.matmul(
                            fp[:, :],
                            wproj_bf[:, h, :],
                            attnT_all[:, h, sl],
                            start=(h == 0),
                            stop=(h == H - 1),
                        )
                    nc.vector.tensor_scalar_add(fin_sb[:, sl], fp[:, :], bproj_sb[:, :])

                # back to token-major + DMA out
                out_sb = outpool.tile([128, NT_B, 128], F32, tag="out_sb")
                for half in range(2):
                    otp = bank_psum.tile([128, 512], F32, tag="bank")
                    for q in range(4):
                        c = half * 4 + q
                        nc.tensor.transpose(
                            otp[:, q * 128 : (q + 1) * 128],
                            fin_sb[:, c * 128 : (c + 1) * 128],
                            ident_f32[:, :],
                        )
                    nc.vector.tensor_copy(
                        out_sb[:, half * 4 : (half + 1) * 4, :], otp[:, :].rearrange("p (c e) -> p c e", e=128)
                    )
                nc.sync.dma_start(
                    out_ext[b, :, :].rearrange("(c p) e -> p c e", p=128),
                    out_sb[:, :, :],
                )

    split_sync_waits(nc, max_waits=1)
    return nc


_CACHED = {}


def _get_nc():
    if "nc" not in _CACHED:
        _CACHED["nc"] = build_nc()
    return _CACHED["nc"]


def run(inputs, trace=False, trace_kwargs=None):
    """inputs: full-problem dict as from setup_inputs(). Returns (out, results)."""
    x = np.ascontiguousarray(np.asarray(inputs["inputs"], dtype=np.float32))
    shared = {
        "ln_gamma": np.ascontiguousarray(np.asarray(inputs["ln_gamma"], np.float32)),
        "ln_beta": np.ascontiguousarray(np.asarray(inputs["ln_beta"], np.float32)),
        "w_qkv": np.ascontiguousarray(np.asarray(inputs["w_qkv"], np.float32)),
        "w_proj": np.ascontiguousarray(np.asarray(inputs["w_proj"], np.float32)),
        "b_proj": np.ascontiguousarray(np.asarray(inputs["b_proj"], np.float32)),
    }
    in_maps = []
    for i in range(N_CORES):
        m = dict(shared)
        m["x"] = np.ascontiguousarray(x[i * B_LOC : (i + 1) * B_LOC])
        in_maps.append(m)

    nc = _get_nc()
    kw = {}
    if trace:
        kw["trace"] = True
        if trace_kwargs:
            kw["trace_kwargs"] = trace_kwargs
    res = run_bass_kernel_spmd(nc, in_maps, list(range(N_CORES)), **kw)
    out = np.concatenate([res.results[i]["out"] for i in range(N_CORES)], axis=0)
    return out, res


def kernel(**inputs) -> np.ndarray:
    out, _ = run(inputs)
    return out
